# revision 17
# baseline (speedup 1.0000x reference)
"""Bottleneck-transformer block on 8 TRN2 NeuronCores — fp8 DoubleRow version.

Sharding: data-parallel over batch (B=64 -> 8 elements/core), weights
replicated; no collectives. All GEMMs run as fp8e4 DoubleRow matmuls
(K=256 per instruction at 0.5 cycles/row): conv1, q/k/v projections,
attention logits (rel-pos term packed into the second DR plane), attn@V,
and conv3+shortcut fused into one PSUM accumulation group per output tile.
The shortcut GEMM dominates the fp8 error budget, so it is hi/lo
error-compensated (w_hi*x_hi + w_lo*x_hi + w_hi*x_lo at one shared scale).

BatchNorms, conv biases, and the attention value bias fold on the host; q/k
biases only shift logits by per-column constants under softmax and are
dropped / merged exactly by using the biased q-hat as the logits rhs plane.
Per-output-channel weight scales dequantize through per-partition ACT/DVE
scale+bias vectors. Softmax column sums run as GPSIMD partition all-reduces
(PE/ACT stay out of the chain). The final conv PSUM is DMA'd raw to DRAM;
the last affine+relu is applied on the host during unsharding.

Pipelining: element e's final conv is interleaved into element e+1's
trunk/attention emission so the PE never idles (keeps the tensor engine's
p-state ramp warm); x tiles are prefetched two elements ahead so input DMA
is not queued behind output DMA.
"""

import numpy as np
import ml_dtypes

import concourse.bass as bass
import concourse.mybir as mybir
from concourse import bacc
from concourse.bass_isa import ReduceOp
from concourse.tile import TileContext
from concourse.bass_utils import run_bass_kernel_spmd

EPS = 1e-5
NCORES = 8
BLOC = 8           # batch elements per core
NT = 256           # tokens per element (16*16)
F32 = mybir.dt.float32
F16 = mybir.dt.float16
F8 = mybir.dt.float8e4
E4 = ml_dtypes.float8_e4m3
DR = mybir.MatmulPerfMode.DoubleRow
MULT = mybir.AluOpType.mult
ADD = mybir.AluOpType.add

# quantization design constants (input distribution is fixed by the problem)
SX = 16.0          # x activation scale
SO1 = 16.0         # out1 activation scale
SQ = 32.0          # q scale
SKP = 32.0         # k / rel-pos shared scale
SO2 = 16.0         # out2 activation scale
SAFE = 0.95
MAXV = 224.0
EXPSCALE = 1.0 / (SQ * SKP)
EXPSHIFT = float(np.log(64.0) - 4.85)

_STATE = {}

_F8_SHAPES = {
    "xh": [BLOC, 128, 4, 2, NT], "xl": [BLOC, 128, 4, 2, NT],
    "w1": [128, 4, 2, 512],
    "qw": [128, 2, 2, 512], "kw": [128, 2, 2, 512], "vw": [128, 2, 2, 512],
    "w3": [128, 2, 2, 2048],
    "wsh": [128, 4, 2, 2048], "wsl": [128, 4, 2, 2048],
    "pos": [128, 4, NT],
}
_F32_SHAPES = {
    "b1s": [128, 4], "b1b": [128, 4], "qs": [128, 4], "qb8": [128, 4],
    "ks": [128, 4], "o2s": [128, 4], "o2b": [128, 4], "uvt": [128, 1],
}


def _build_nc():
    nc = bacc.Bacc("TRN2", target_bir_lowering=False, debug=False,
                   num_devices=NCORES)
    d = {}
    for k, v in _F8_SHAPES.items():
        d[k] = nc.declare_dram_parameter(k, v, F8, isOutput=False)
    for k, v in _F32_SHAPES.items():
        d[k] = nc.declare_dram_parameter(k, v, F32, isOutput=False)
    out_d = nc.declare_dram_parameter("out", [BLOC, 8, 128, 2, NT], F16,
                                      isOutput=True)

    RELU = mybir.ActivationFunctionType.Relu
    EXPF = mybir.ActivationFunctionType.Exp
    COPY = mybir.ActivationFunctionType.Copy
    RADD = ReduceOp.add

    def mmdr(ps, lhsT, rhs, start, stop):
        nc.tensor.matmul(ps, lhsT, rhs, start=start, stop=stop, perf_mode=DR)

    with TileContext(nc) as tc:
        with (
            tc.tile_pool(name="wp", bufs=1) as wp,
            tc.tile_pool(name="act", bufs=2) as act,
            tc.tile_pool(name="att", bufs=4) as att,
            tc.tile_pool(name="psA", bufs=4, space="PSUM") as psA,
        ):
            W1 = wp.tile([128, 4, 2, 512], F8)
            QW = wp.tile([128, 2, 2, 512], F8)
            KW = wp.tile([128, 2, 2, 512], F8)
            VW = wp.tile([128, 2, 2, 512], F8)
            W3 = wp.tile([128, 2, 2, 2048], F8)
            WSH = wp.tile([128, 4, 2, 2048], F8)
            WSL = wp.tile([128, 4, 2, 2048], F8)
            PR = wp.tile([128, 4, 2, NT], F8)    # plane0: q-hat, plane1: pos
            ESH = wp.tile([128, 1], F32)
            ESC = wp.tile([128, 1], F32)
            B1S = wp.tile([128, 4], F32)
            B1B = wp.tile([128, 4], F32)
            QS = wp.tile([128, 4], F32)
            QB8 = wp.tile([128, 4], F32)
            KS = wp.tile([128, 4], F32)
            O2S = wp.tile([128, 4], F32)
            O2B = wp.tile([128, 4], F32)
            UVT = wp.tile([128, 1], F32)

            nc.vector.memset(ESH, EXPSHIFT)
            nc.vector.memset(ESC, EXPSCALE)
            FSC = wp.tile([128, 1], F32)
            nc.vector.memset(FSC, 1.0 / 256.0)

            def fetch_x(e, defer_xl=False):
                XH = act.tile([128, 4, 2, NT], F8, tag="xh", bufs=3,
                              name=f"xh{e}")
                XL = act.tile([128, 4, 2, NT], F8, tag="xl", bufs=3,
                              name=f"xl{e}")
                nc.sync.dma_start(out=XH, in_=d["xh"][e])
                if not defer_xl:
                    nc.sync.dma_start(out=XL, in_=d["xl"][e])
                return XH, XL

            def load_vecs():
                for k, t in [("b1s", B1S), ("b1b", B1B), ("qs", QS),
                             ("qb8", QB8), ("ks", KS), ("o2s", O2S),
                             ("o2b", O2B), ("uvt", UVT)]:
                    nc.sync.dma_start(out=t, in_=d[k][:])

            def load_small_weights():
                nc.sync.dma_start(out=QW, in_=d["qw"][:])
                nc.sync.dma_start(out=KW, in_=d["kw"][:])
                nc.sync.dma_start(out=VW, in_=d["vw"][:])
                for h in range(4):
                    nc.sync.dma_start(out=PR[:, h, 1, :], in_=d["pos"][:, h, :])

            def load_big_weights():
                for g in range(4):
                    sl = slice(g * 512, (g + 1) * 512)
                    nc.sync.dma_start(out=WSH[:, :, :, sl],
                                      in_=d["wsh"][:, :, :, sl])
                    nc.sync.dma_start(out=WSL[:, :, :, sl],
                                      in_=d["wsl"][:, :, :, sl])
                    nc.sync.dma_start(out=W3[:, :, :, sl],
                                      in_=d["w3"][:, :, :, sl])

            def final_chunk(st, mps):
                """conv3 + compensated shortcut for m-tile PAIRS mps of the
                element st = (e, XH, XL, O2): one [128,512] single-group psum
                per pair, copied raw to fp16 SBUF (ACT/DVE alternating) and
                DMA'd out; the final affine+relu runs on the host."""
                if st is None:
                    return
                e, XH, XL, O2 = st
                for mp in mps:
                    ps = psA.tile([128, 2, NT], F32, tag="mmf", bufs=3)
                    first = True
                    for half in range(2):
                        sl = slice((2 * mp + half) * 128,
                                   (2 * mp + half + 1) * 128)
                        pt = ps[:, half, :]
                        for jj in range(4):
                            nc.tensor.matmul(pt, WSH[:, jj, :, sl], XH[:, jj],
                                             start=first, stop=False,
                                             perf_mode=DR,
                                             skip_group_check=True)
                            first = False
                        for jj in range(4):
                            nc.tensor.matmul(pt, WSL[:, jj, :, sl], XH[:, jj],
                                             start=False, stop=False,
                                             perf_mode=DR,
                                             skip_group_check=True)
                        for jj in range(4):
                            nc.tensor.matmul(pt, WSH[:, jj, :, sl], XL[:, jj],
                                             start=False, stop=False,
                                             perf_mode=DR,
                                             skip_group_check=True)
                        for jj in range(2):
                            nc.tensor.matmul(pt, W3[:, jj, :, sl], O2[:, jj],
                                             start=False,
                                             stop=(half == 1 and jj == 1),
                                             perf_mode=DR,
                                             skip_group_check=True)
                    ot = act.tile([128, 2, NT], F16, tag="ot", bufs=4,
                                  name=f"ot{e}_{mp}")
                    if mp % 2 == 0:
                        nc.scalar.activation(ot, ps, COPY, bias=0.0,
                                             scale=FSC)
                    else:
                        nc.vector.tensor_scalar_mul(out=ot, in0=ps,
                                                    scalar1=FSC)
                    nc.sync.dma_start(out=out_d[e, mp], in_=ot)

            def trunk(e, XH, XL, prev):
                """conv1 + q/k/v + logits/exp for elem e, interleaved with
                the first half of prev's final conv."""
                final_chunk(prev, range(0, 1))
                O1 = act.tile([128, 2, 2, NT], F8, tag="o1", name=f"o1_{e}")
                for m in range(4):
                    ps = psA.tile([128, NT], F32, tag="mm")
                    for jj in range(4):
                        mmdr(ps, W1[:, jj, :, m * 128:(m + 1) * 128],
                             XH[:, jj], jj == 0, jj == 3)
                    nc.scalar.activation(O1[:, m // 2, m % 2, :], ps, RELU,
                                         bias=B1B[:, m:m + 1],
                                         scale=B1S[:, m:m + 1])
                final_chunk(prev, range(1, 2))

                # q projection: raw into KQ plane1 (ACT), q-hat into PR (DVE)
                KQ = act.tile([128, 4, 2, NT], F8, tag="kq", name=f"kq{e}")
                for h in range(4):
                    ps = psA.tile([128, NT], F32, tag="mm")
                    for jj in range(2):
                        mmdr(ps, QW[:, jj, :, h * 128:(h + 1) * 128],
                             O1[:, jj], jj == 0, jj == 1)
                    nc.scalar.activation(KQ[:, h, 1, :], ps, COPY,
                                         bias=0.0, scale=QS[:, h:h + 1])
                    nc.vector.tensor_scalar(out=PR[:, h, 0, :], in0=ps,
                                            scalar1=QS[:, h:h + 1],
                                            scalar2=QB8[:, h:h + 1],
                                            op0=MULT, op1=ADD)
                final_chunk(prev, range(2, 3))

                # k projection (no bias needed) into KQ plane0 (ACT)
                for h in range(4):
                    ps = psA.tile([128, NT], F32, tag="mm")
                    for jj in range(2):
                        mmdr(ps, KW[:, jj, :, h * 128:(h + 1) * 128],
                             O1[:, jj], jj == 0, jj == 1)
                    nc.vector.tensor_scalar_mul(out=KQ[:, h, 0, :], in0=ps,
                                                scalar1=KS[:, h:h + 1])
                final_chunk(prev, range(3, 4))

                # v, transposed: VT[tok, mt, c] (per-channel svw rides along)
                VT = act.tile([128, 2, 512], F8, tag="vt", name=f"vt{e}")
                for mt in range(2):
                    for cc in range(2):
                        ps = psA.tile([128, NT], F32, tag="mm")
                        for jj in range(2):
                            mmdr(ps, O1[:, jj, :, mt * 128:(mt + 1) * 128],
                                 VW[:, jj, :, cc * 256:(cc + 1) * 256],
                                 jj == 0, jj == 1)
                        nc.vector.tensor_scalar_mul(
                            out=VT[:, mt, cc * 256:(cc + 1) * 256], in0=ps,
                            scalar1=UVT)
                final_chunk(prev, range(4, 5))

                # logits^T + exp: one 512-wide psum group per head
                EXT = att.tile([128, 4, 2, NT], F8, tag="ext", bufs=2,
                               name=f"ext{e}")
                for h in range(4):
                    psl = psA.tile([128, 2, NT], F32, tag="mml", bufs=1)
                    for mt in range(2):
                        nc.tensor.matmul(
                            psl[:, mt, :],
                            KQ[:, h, :, mt * 128:(mt + 1) * 128], PR[:, h],
                            start=(mt == 0), stop=(mt == 1), perf_mode=DR,
                            skip_group_check=True)
                    nc.scalar.activation(EXT[:, h], psl, EXPF,
                                         bias=ESH, scale=ESC)
                return VT, EXT

            def attn(e, VT, EXT, prev):
                """softmax + attn@V for elem e, interleaved with the second
                half of prev's final conv."""
                O2 = act.tile([128, 2, 2, NT], F8, tag="o2", name=f"o2_{e}")
                for h in range(4):
                    ALR = att.tile([128, 2, NT], F32, tag="alr", bufs=4,
                                   name=f"alr{e}_{h}")
                    nc.gpsimd.partition_all_reduce(ALR, EXT[:, h], 128, RADD)
                    SUM = att.tile([128, NT], F32, tag="sumb", bufs=4,
                                   name=f"sum{e}_{h}")
                    nc.gpsimd.tensor_tensor(out=SUM, in0=ALR[:, 0, :],
                                            in1=ALR[:, 1, :], op=ADD)
                    RCP = att.tile([128, NT], F32, tag="rcp", bufs=4,
                                   name=f"rcp{e}_{h}")
                    with nc.allow_low_precision(reason="softmax 1/sum"):
                        nc.vector.reciprocal(out=RCP, in_=SUM)
                    final_chunk(prev, range(5 + h, 6 + h) if h < 3 else [])
                    pso = psA.tile([128, NT], F32, tag="mm")
                    mmdr(pso, VT[:, :, h * 128:(h + 1) * 128], EXT[:, h],
                         True, True)
                    tmp = att.tile([128, NT], F32, tag="tmp", bufs=2,
                                   name=f"tmp{e}_{h}")
                    nc.vector.tensor_tensor(out=tmp, in0=pso,
                                            in1=RCP, op=MULT)
                    nc.scalar.activation(O2[:, h // 2, h % 2, :], tmp, RELU,
                                         bias=O2B[:, h:h + 1],
                                         scale=O2S[:, h:h + 1])
                return O2

            # startup: W1 + first x land first so conv1 starts immediately
            nc.sync.dma_start(out=W1[:, :, :, 0:128],
                              in_=d["w1"][:, :, :, 0:128])
            xq = [fetch_x(0, defer_xl=True)]
            for g in range(1, 4):
                sl = slice(g * 128, (g + 1) * 128)
                nc.sync.dma_start(out=W1[:, :, :, sl],
                                  in_=d["w1"][:, :, :, sl])
            load_vecs()
            load_small_weights()
            nc.sync.dma_start(out=xq[0][1], in_=d["xl"][0])
            xq.append(fetch_x(1))
            load_big_weights()

            prev = None
            for e in range(BLOC):
                XH, XL = xq[e]
                VT, EXT = trunk(e, XH, XL, prev)
                if e + 2 < BLOC:
                    xq.append(fetch_x(e + 2))
                O2 = attn(e, VT, EXT, prev)
                prev = (e, XH, XL, O2)
            final_chunk(prev, range(8))

    nc.compile()
    return nc


def _q8(x):
    """quantize to fp8e4 values, returned as float32."""
    return np.asarray(x, np.float32).astype(E4).astype(np.float32)


def _r8(wq):
    """quantized [M, K] weight (f32 container) -> lhsT/moving layout
    [128, K//256, 2, M] fp8."""
    m, k = wq.shape
    t = np.ascontiguousarray(
        wq.T.reshape(k // 256, 2, 128, m).transpose(2, 0, 1, 3))
    return t.astype(E4)


def _b(v):
    """[C] vector -> [128, C//128] per-m-tile layout."""
    return np.ascontiguousarray(
        np.asarray(v, np.float64).reshape(-1, 128).T).astype(np.float32)


def _prep_shared(i):
    s1 = (i["bn1_g"] / np.sqrt(i["bn1_v"] + EPS)).astype(np.float64)
    w1f = i["conv1_w"].astype(np.float64) * s1[:, None]
    b1 = i["bn1_b"].astype(np.float64) - i["bn1_m"].astype(np.float64) * s1

    s2 = (i["bn2_g"] / np.sqrt(i["bn2_v"] + EPS)).astype(np.float64)
    b2 = (i["bn2_b"].astype(np.float64)
          - i["bn2_m"].astype(np.float64) * s2
          + s2 * i["v_b"].astype(np.float64))   # v bias folded (probs sum 1)

    s3 = (i["bn3_g"] / np.sqrt(i["bn3_v"] + EPS)).astype(np.float64)
    w3f = i["conv3_w"].astype(np.float64) * s3[:, None]
    b3 = i["bn3_b"].astype(np.float64) - i["bn3_m"].astype(np.float64) * s3

    ss = (i["scbn_g"] / np.sqrt(i["scbn_v"] + EPS)).astype(np.float64)
    wscf = i["sc_w"].astype(np.float64) * ss[:, None]
    bsc = (ss * (i["sc_b"].astype(np.float64)
                 - i["scbn_m"].astype(np.float64))
           + i["scbn_b"].astype(np.float64))

    sw1 = SAFE * MAXV / np.abs(w1f).max(axis=1)
    sqw = SAFE * MAXV / np.abs(i["q_w"]).max(axis=1)
    skw = SAFE * MAXV / np.abs(i["k_w"]).max(axis=1)
    svw = SAFE * MAXV / np.abs(i["v_w"]).max(axis=1)
    c_ch = SAFE * MAXV / np.maximum(np.abs(w3f).max(axis=1) / SO2,
                                    np.abs(wscf).max(axis=1) / SX)
    uv = 32.0 / (SO1 * np.median(svw))

    wscs = wscf * (c_ch / SX)[:, None]
    wsch_f = _q8(wscs)

    pos = (np.asarray(i["rel_h"], np.float64)
           + np.asarray(i["rel_w"], np.float64)).reshape(4, 128, NT)
    pos8 = (pos * SKP).astype(np.float32).astype(E4)

    dev = {
        "w1": _r8(_q8(w1f * sw1[:, None])),
        "qw": _r8(_q8(i["q_w"] * sqw[:, None])),
        "kw": _r8(_q8(i["k_w"] * skw[:, None])),
        "vw": _r8(_q8(i["v_w"] * svw[:, None])),
        "w3": _r8(_q8(w3f * (c_ch / SO2)[:, None])),
        "wsh": _r8(wsch_f),
        "wsl": _r8(_q8(wscs - wsch_f)),
        "pos": np.ascontiguousarray(pos8.transpose(1, 0, 2)),
        "b1s": _b(1.0 / (sw1 * SX) * SO1), "b1b": _b(b1 * SO1),
        "qs": _b(SQ / (sqw * SO1)), "qb8": _b(SQ * i["q_b"]),
        "ks": _b(SKP / (skw * SO1)),
        "o2s": _b(s2 * SO2 / (svw * SO1 * uv)), "o2b": _b(b2 * SO2),
        "uvt": np.full((128, 1), uv, np.float32),
    }
    # host-side final affine+relu coefficients (channel-major [2048, 1])
    host = {
        "fs": (256.0 / c_ch).astype(np.float32).reshape(2048, 1),
        "fb": (b3 + bsc).astype(np.float32).reshape(2048, 1),
    }
    return dev, host


def _prep_x(x):
    """full x [64, 1024, 16, 16] -> per-core hi/lo fp8 [8, BLOC,128,4,2,NT]."""
    xs = np.asarray(x, np.float32).reshape(64, 1024, NT) * SX
    xh = xs.astype(E4)
    xl = (xs - xh.astype(np.float32)).astype(E4)

    def lay(a):
        t = a.reshape(64, 4, 2, 128, NT).transpose(0, 3, 1, 2, 4)
        return np.ascontiguousarray(t).reshape(8, BLOC, 128, 4, 2, NT)

    return lay(xh), lay(xl)


def kernel(**inputs):
    if "nc" not in _STATE:
        _STATE["nc"] = _build_nc()
    nc = _STATE["nc"]

    i = {k: np.asarray(v) for k, v in inputs.items()}
    dev, host = _prep_shared(i)
    xh, xl = _prep_x(i["x"])

    in_maps = []
    for c in range(NCORES):
        m = dict(dev)
        m["xh"] = xh[c]
        m["xl"] = xl[c]
        in_maps.append(m)

    res = run_bass_kernel_spmd(nc, in_maps, list(range(NCORES)))
    raw = np.concatenate(
        [res.results[c]["out"].astype(np.float32)
         .transpose(0, 1, 3, 2, 4).reshape(BLOC, 2048, NT)
         for c in range(NCORES)], axis=0)
    out = np.maximum(raw * host["fs"][None] + host["fb"][None], 0.0)
    return out.reshape(64, 2048, 16, 16).astype(np.float32)


# revision 18
# speedup vs baseline: 1.0136x; 1.0136x over previous
"""Bottleneck-transformer block on 8 TRN2 NeuronCores — fp8 DoubleRow version.

Sharding: data-parallel over batch (B=64 -> 8 elements/core), weights
replicated; no collectives. All GEMMs run as fp8e4 DoubleRow matmuls
(K=256 per instruction at 0.5 cycles/row): conv1, q/k/v projections,
attention logits (rel-pos term packed into the second DR plane), attn@V,
and conv3+shortcut fused into one PSUM accumulation group per output tile.
The shortcut GEMM dominates the fp8 error budget, so it is hi/lo
error-compensated (w_hi*x_hi + w_lo*x_hi + w_hi*x_lo at one shared scale).

BatchNorms, conv biases, and the attention value bias fold on the host; q/k
biases only shift logits by per-column constants under softmax and are
dropped / merged exactly by using the biased q-hat as the logits rhs plane.
Per-output-channel weight scales dequantize through per-partition ACT/DVE
scale+bias vectors. Softmax column sums run as GPSIMD partition all-reduces
(PE/ACT stay out of the chain). The final conv PSUM is DMA'd raw to DRAM;
the last affine+relu is applied on the host during unsharding.

Pipelining: element e's final conv is interleaved into element e+1's
trunk/attention emission so the PE never idles (keeps the tensor engine's
p-state ramp warm); x tiles are prefetched two elements ahead so input DMA
is not queued behind output DMA.
"""

import numpy as np
import ml_dtypes

import concourse.bass as bass
import concourse.mybir as mybir
from concourse import bacc
from concourse.bass_isa import ReduceOp
from concourse.tile import TileContext
from concourse.bass_utils import run_bass_kernel_spmd

EPS = 1e-5
NCORES = 8
BLOC = 8           # batch elements per core
NT = 256           # tokens per element (16*16)
F32 = mybir.dt.float32
F16 = mybir.dt.float16
F8 = mybir.dt.float8e4
E4 = ml_dtypes.float8_e4m3
DR = mybir.MatmulPerfMode.DoubleRow
MULT = mybir.AluOpType.mult
ADD = mybir.AluOpType.add

# quantization design constants (input distribution is fixed by the problem)
SX = 16.0          # x activation scale
SO1 = 16.0         # out1 activation scale
SQ = 32.0          # q scale
SKP = 32.0         # k / rel-pos shared scale
SO2 = 16.0         # out2 activation scale
SAFE = 0.95
MAXV = 224.0
EXPSCALE = 1.0 / (SQ * SKP)
EXPSHIFT = float(np.log(64.0) - 4.85)

_STATE = {}

_F8_SHAPES = {
    "xh": [BLOC, 128, 4, 2, NT], "xl": [BLOC, 128, 4, 2, NT],
    "w1": [128, 4, 2, 512],
    "qw": [128, 2, 2, 512], "kw": [128, 2, 2, 512], "vw": [128, 2, 2, 512],
    "w3": [128, 2, 2, 2048],
    "wsh": [128, 4, 2, 2048], "wsl": [128, 4, 2, 2048],
    "pos": [128, 4, NT],
}
_F32_SHAPES = {
    "b1s": [128, 4], "b1b": [128, 4], "qs": [128, 4], "qb8": [128, 4],
    "ks": [128, 4], "o2s": [128, 4], "o2b": [128, 4], "uvt": [128, 1],
}


def _build_nc():
    nc = bacc.Bacc("TRN2", target_bir_lowering=False, debug=False,
                   num_devices=NCORES)
    d = {}
    for k, v in _F8_SHAPES.items():
        d[k] = nc.declare_dram_parameter(k, v, F8, isOutput=False)
    for k, v in _F32_SHAPES.items():
        d[k] = nc.declare_dram_parameter(k, v, F32, isOutput=False)
    out_d = nc.declare_dram_parameter("out", [BLOC, 8, 128, 2, NT], F16,
                                      isOutput=True)

    RELU = mybir.ActivationFunctionType.Relu
    EXPF = mybir.ActivationFunctionType.Exp
    COPY = mybir.ActivationFunctionType.Copy
    RADD = ReduceOp.add

    def mmdr(ps, lhsT, rhs, start, stop):
        nc.tensor.matmul(ps, lhsT, rhs, start=start, stop=stop, perf_mode=DR)

    with TileContext(nc) as tc:
        with (
            tc.tile_pool(name="wp", bufs=1) as wp,
            tc.tile_pool(name="act", bufs=2) as act,
            tc.tile_pool(name="att", bufs=4) as att,
            tc.tile_pool(name="psA", bufs=4, space="PSUM") as psA,
        ):
            W1 = wp.tile([128, 4, 2, 512], F8)
            QW = wp.tile([128, 2, 2, 512], F8)
            KW = wp.tile([128, 2, 2, 512], F8)
            VW = wp.tile([128, 2, 2, 512], F8)
            W3 = wp.tile([128, 2, 2, 2048], F8)
            WSH = wp.tile([128, 4, 2, 2048], F8)
            WSL = wp.tile([128, 4, 2, 2048], F8)
            PR = wp.tile([128, 4, 2, NT], F8)    # plane0: q-hat, plane1: pos
            ESH = wp.tile([128, 1], F32)
            ESC = wp.tile([128, 1], F32)
            B1S = wp.tile([128, 4], F32)
            B1B = wp.tile([128, 4], F32)
            QS = wp.tile([128, 4], F32)
            QB8 = wp.tile([128, 4], F32)
            KS = wp.tile([128, 4], F32)
            O2S = wp.tile([128, 4], F32)
            O2B = wp.tile([128, 4], F32)
            UVT = wp.tile([128, 1], F32)

            nc.vector.memset(ESH, EXPSHIFT)
            nc.vector.memset(ESC, EXPSCALE)
            FSC = wp.tile([128, 1], F32)
            nc.vector.memset(FSC, 1.0 / 256.0)

            def fetch_x(e, defer_xl=False):
                XH = act.tile([128, 4, 2, NT], F8, tag="xh", bufs=3,
                              name=f"xh{e}")
                XL = act.tile([128, 4, 2, NT], F8, tag="xl", bufs=3,
                              name=f"xl{e}")
                nc.sync.dma_start(out=XH, in_=d["xh"][e])
                if not defer_xl:
                    nc.sync.dma_start(out=XL, in_=d["xl"][e])
                return XH, XL

            def load_vecs():
                for k, t in [("b1s", B1S), ("b1b", B1B), ("qs", QS),
                             ("qb8", QB8), ("ks", KS), ("o2s", O2S),
                             ("o2b", O2B), ("uvt", UVT)]:
                    nc.sync.dma_start(out=t, in_=d[k][:])

            def load_small_weights():
                nc.sync.dma_start(out=QW, in_=d["qw"][:])
                nc.sync.dma_start(out=KW, in_=d["kw"][:])
                nc.sync.dma_start(out=VW, in_=d["vw"][:])
                for h in range(4):
                    nc.sync.dma_start(out=PR[:, h, 1, :], in_=d["pos"][:, h, :])

            def load_big_weights():
                for g in range(4):
                    sl = slice(g * 512, (g + 1) * 512)
                    nc.sync.dma_start(out=WSH[:, :, :, sl],
                                      in_=d["wsh"][:, :, :, sl])
                    nc.sync.dma_start(out=WSL[:, :, :, sl],
                                      in_=d["wsl"][:, :, :, sl])
                    nc.sync.dma_start(out=W3[:, :, :, sl],
                                      in_=d["w3"][:, :, :, sl])

            def final_chunk(st, mps):
                """conv3 + compensated shortcut for m-tile PAIRS mps of the
                element st = (e, XH, XL, O2): one [128,512] single-group psum
                per pair, copied raw to fp16 SBUF (ACT/DVE alternating) and
                DMA'd out; the final affine+relu runs on the host."""
                if st is None:
                    return
                e, XH, XL, O2 = st
                for mp in mps:
                    ps = psA.tile([128, 2, NT], F32, tag="mmf", bufs=3)
                    first = True
                    for half in range(2):
                        sl = slice((2 * mp + half) * 128,
                                   (2 * mp + half + 1) * 128)
                        pt = ps[:, half, :]
                        for jj in range(4):
                            nc.tensor.matmul(pt, WSH[:, jj, :, sl], XH[:, jj],
                                             start=first, stop=False,
                                             perf_mode=DR,
                                             skip_group_check=True)
                            first = False
                        for jj in range(4):
                            nc.tensor.matmul(pt, WSL[:, jj, :, sl], XH[:, jj],
                                             start=False, stop=False,
                                             perf_mode=DR,
                                             skip_group_check=True)
                        for jj in range(4):
                            nc.tensor.matmul(pt, WSH[:, jj, :, sl], XL[:, jj],
                                             start=False, stop=False,
                                             perf_mode=DR,
                                             skip_group_check=True)
                        for jj in range(2):
                            nc.tensor.matmul(pt, W3[:, jj, :, sl], O2[:, jj],
                                             start=False,
                                             stop=(half == 1 and jj == 1),
                                             perf_mode=DR,
                                             skip_group_check=True)
                    ot = act.tile([128, 2, NT], F16, tag="ot", bufs=4,
                                  name=f"ot{e}_{mp}")
                    if mp % 2 == 0:
                        nc.scalar.activation(ot, ps, COPY, bias=0.0,
                                             scale=FSC)
                    else:
                        nc.vector.tensor_scalar_mul(out=ot, in0=ps,
                                                    scalar1=FSC)
                    nc.sync.dma_start(out=out_d[e, mp], in_=ot)

            def trunk(e, XH, XL, prev):
                """conv1 + q/k/v + logits/exp for elem e, interleaved with
                the first half of prev's final conv."""
                final_chunk(prev, range(0, 1))
                O1 = act.tile([128, 2, 2, NT], F8, tag="o1", name=f"o1_{e}")
                for m in range(4):
                    ps = psA.tile([128, NT], F32, tag="mm")
                    for jj in range(4):
                        mmdr(ps, W1[:, jj, :, m * 128:(m + 1) * 128],
                             XH[:, jj], jj == 0, jj == 3)
                    nc.scalar.activation(O1[:, m // 2, m % 2, :], ps, RELU,
                                         bias=B1B[:, m:m + 1],
                                         scale=B1S[:, m:m + 1])

                # q projection: raw into KQ plane1 (ACT), q-hat into PR (DVE)
                KQ = act.tile([128, 4, 2, NT], F8, tag="kq", name=f"kq{e}")
                for h in range(4):
                    ps = psA.tile([128, NT], F32, tag="mm")
                    for jj in range(2):
                        mmdr(ps, QW[:, jj, :, h * 128:(h + 1) * 128],
                             O1[:, jj], jj == 0, jj == 1)
                    nc.scalar.activation(KQ[:, h, 1, :], ps, COPY,
                                         bias=0.0, scale=QS[:, h:h + 1])
                    nc.vector.tensor_scalar(out=PR[:, h, 0, :], in0=ps,
                                            scalar1=QS[:, h:h + 1],
                                            scalar2=QB8[:, h:h + 1],
                                            op0=MULT, op1=ADD)
                final_chunk(prev, range(1, 2))

                # k projection (no bias needed) into KQ plane0 (ACT)
                for h in range(4):
                    ps = psA.tile([128, NT], F32, tag="mm")
                    for jj in range(2):
                        mmdr(ps, KW[:, jj, :, h * 128:(h + 1) * 128],
                             O1[:, jj], jj == 0, jj == 1)
                    nc.vector.tensor_scalar_mul(out=KQ[:, h, 0, :], in0=ps,
                                                scalar1=KS[:, h:h + 1])
                final_chunk(prev, range(2, 3))

                # v, transposed: VT[tok, mt, c] (per-channel svw rides along)
                VT = act.tile([128, 2, 512], F8, tag="vt", name=f"vt{e}")
                for mt in range(2):
                    for cc in range(2):
                        ps = psA.tile([128, NT], F32, tag="mm")
                        for jj in range(2):
                            mmdr(ps, O1[:, jj, :, mt * 128:(mt + 1) * 128],
                                 VW[:, jj, :, cc * 256:(cc + 1) * 256],
                                 jj == 0, jj == 1)
                        nc.vector.tensor_scalar_mul(
                            out=VT[:, mt, cc * 256:(cc + 1) * 256], in0=ps,
                            scalar1=UVT)
                final_chunk(prev, range(3, 4))

                # logits^T + exp: one 512-wide psum group per head
                EXT = att.tile([128, 4, 2, NT], F8, tag="ext", bufs=2,
                               name=f"ext{e}")
                for h in range(4):
                    psl = psA.tile([128, 2, NT], F32, tag="mml", bufs=1)
                    for mt in range(2):
                        nc.tensor.matmul(
                            psl[:, mt, :],
                            KQ[:, h, :, mt * 128:(mt + 1) * 128], PR[:, h],
                            start=(mt == 0), stop=(mt == 1), perf_mode=DR,
                            skip_group_check=True)
                    nc.scalar.activation(EXT[:, h], psl, EXPF,
                                         bias=ESH, scale=ESC)
                return VT, EXT

            def attn(e, VT, EXT, prev):
                """softmax + attn@V for elem e, interleaved with the second
                half of prev's final conv."""
                O2 = act.tile([128, 2, 2, NT], F8, tag="o2", name=f"o2_{e}")
                for h in range(4):
                    ALR = att.tile([128, 2, NT], F32, tag="alr", bufs=4,
                                   name=f"alr{e}_{h}")
                    nc.gpsimd.partition_all_reduce(ALR, EXT[:, h], 128, RADD)
                    SUM = att.tile([128, NT], F32, tag="sumb", bufs=4,
                                   name=f"sum{e}_{h}")
                    nc.gpsimd.tensor_tensor(out=SUM, in0=ALR[:, 0, :],
                                            in1=ALR[:, 1, :], op=ADD)
                    RCP = att.tile([128, NT], F32, tag="rcp", bufs=4,
                                   name=f"rcp{e}_{h}")
                    with nc.allow_low_precision(reason="softmax 1/sum"):
                        nc.vector.reciprocal(out=RCP, in_=SUM)
                    final_chunk(prev, range(4 + h, 5 + h))
                    pso = psA.tile([128, NT], F32, tag="mm")
                    mmdr(pso, VT[:, :, h * 128:(h + 1) * 128], EXT[:, h],
                         True, True)
                    tmp = att.tile([128, NT], F32, tag="tmp", bufs=2,
                                   name=f"tmp{e}_{h}")
                    nc.vector.tensor_tensor(out=tmp, in0=pso,
                                            in1=RCP, op=MULT)
                    nc.scalar.activation(O2[:, h // 2, h % 2, :], tmp, RELU,
                                         bias=O2B[:, h:h + 1],
                                         scale=O2S[:, h:h + 1])
                return O2

            # startup: W1 + first x land first so conv1 starts immediately
            nc.sync.dma_start(out=W1[:, :, :, 0:128],
                              in_=d["w1"][:, :, :, 0:128])
            xq = [fetch_x(0, defer_xl=True)]
            for g in range(1, 4):
                sl = slice(g * 128, (g + 1) * 128)
                nc.sync.dma_start(out=W1[:, :, :, sl],
                                  in_=d["w1"][:, :, :, sl])
            load_vecs()
            load_small_weights()
            nc.sync.dma_start(out=xq[0][1], in_=d["xl"][0])
            xq.append(fetch_x(1))
            load_big_weights()

            prev = None
            for e in range(BLOC):
                XH, XL = xq[e]
                VT, EXT = trunk(e, XH, XL, prev)
                if e + 2 < BLOC:
                    xq.append(fetch_x(e + 2))
                O2 = attn(e, VT, EXT, prev)
                prev = (e, XH, XL, O2)
            final_chunk(prev, range(8))

    nc.compile()
    return nc


def _q8(x):
    """quantize to fp8e4 values, returned as float32."""
    return np.asarray(x, np.float32).astype(E4).astype(np.float32)


def _r8(wq):
    """quantized [M, K] weight (f32 container) -> lhsT/moving layout
    [128, K//256, 2, M] fp8."""
    m, k = wq.shape
    t = np.ascontiguousarray(
        wq.T.reshape(k // 256, 2, 128, m).transpose(2, 0, 1, 3))
    return t.astype(E4)


def _b(v):
    """[C] vector -> [128, C//128] per-m-tile layout."""
    return np.ascontiguousarray(
        np.asarray(v, np.float64).reshape(-1, 128).T).astype(np.float32)


def _prep_shared(i):
    s1 = (i["bn1_g"] / np.sqrt(i["bn1_v"] + EPS)).astype(np.float64)
    w1f = i["conv1_w"].astype(np.float64) * s1[:, None]
    b1 = i["bn1_b"].astype(np.float64) - i["bn1_m"].astype(np.float64) * s1

    s2 = (i["bn2_g"] / np.sqrt(i["bn2_v"] + EPS)).astype(np.float64)
    b2 = (i["bn2_b"].astype(np.float64)
          - i["bn2_m"].astype(np.float64) * s2
          + s2 * i["v_b"].astype(np.float64))   # v bias folded (probs sum 1)

    s3 = (i["bn3_g"] / np.sqrt(i["bn3_v"] + EPS)).astype(np.float64)
    w3f = i["conv3_w"].astype(np.float64) * s3[:, None]
    b3 = i["bn3_b"].astype(np.float64) - i["bn3_m"].astype(np.float64) * s3

    ss = (i["scbn_g"] / np.sqrt(i["scbn_v"] + EPS)).astype(np.float64)
    wscf = i["sc_w"].astype(np.float64) * ss[:, None]
    bsc = (ss * (i["sc_b"].astype(np.float64)
                 - i["scbn_m"].astype(np.float64))
           + i["scbn_b"].astype(np.float64))

    sw1 = SAFE * MAXV / np.abs(w1f).max(axis=1)
    sqw = SAFE * MAXV / np.abs(i["q_w"]).max(axis=1)
    skw = SAFE * MAXV / np.abs(i["k_w"]).max(axis=1)
    svw = SAFE * MAXV / np.abs(i["v_w"]).max(axis=1)
    c_ch = SAFE * MAXV / np.maximum(np.abs(w3f).max(axis=1) / SO2,
                                    np.abs(wscf).max(axis=1) / SX)
    uv = 32.0 / (SO1 * np.median(svw))

    wscs = wscf * (c_ch / SX)[:, None]
    wsch_f = _q8(wscs)

    pos = (np.asarray(i["rel_h"], np.float64)
           + np.asarray(i["rel_w"], np.float64)).reshape(4, 128, NT)
    pos8 = (pos * SKP).astype(np.float32).astype(E4)

    dev = {
        "w1": _r8(_q8(w1f * sw1[:, None])),
        "qw": _r8(_q8(i["q_w"] * sqw[:, None])),
        "kw": _r8(_q8(i["k_w"] * skw[:, None])),
        "vw": _r8(_q8(i["v_w"] * svw[:, None])),
        "w3": _r8(_q8(w3f * (c_ch / SO2)[:, None])),
        "wsh": _r8(wsch_f),
        "wsl": _r8(_q8(wscs - wsch_f)),
        "pos": np.ascontiguousarray(pos8.transpose(1, 0, 2)),
        "b1s": _b(1.0 / (sw1 * SX) * SO1), "b1b": _b(b1 * SO1),
        "qs": _b(SQ / (sqw * SO1)), "qb8": _b(SQ * i["q_b"]),
        "ks": _b(SKP / (skw * SO1)),
        "o2s": _b(s2 * SO2 / (svw * SO1 * uv)), "o2b": _b(b2 * SO2),
        "uvt": np.full((128, 1), uv, np.float32),
    }
    # host-side final affine+relu coefficients (channel-major [2048, 1])
    host = {
        "fs": (256.0 / c_ch).astype(np.float32).reshape(2048, 1),
        "fb": (b3 + bsc).astype(np.float32).reshape(2048, 1),
    }
    return dev, host


def _prep_x(x):
    """full x [64, 1024, 16, 16] -> per-core hi/lo fp8 [8, BLOC,128,4,2,NT]."""
    xs = np.asarray(x, np.float32).reshape(64, 1024, NT) * SX
    xh = xs.astype(E4)
    xl = (xs - xh.astype(np.float32)).astype(E4)

    def lay(a):
        t = a.reshape(64, 4, 2, 128, NT).transpose(0, 3, 1, 2, 4)
        return np.ascontiguousarray(t).reshape(8, BLOC, 128, 4, 2, NT)

    return lay(xh), lay(xl)


def kernel(**inputs):
    if "nc" not in _STATE:
        _STATE["nc"] = _build_nc()
    nc = _STATE["nc"]

    i = {k: np.asarray(v) for k, v in inputs.items()}
    dev, host = _prep_shared(i)
    xh, xl = _prep_x(i["x"])

    in_maps = []
    for c in range(NCORES):
        m = dict(dev)
        m["xh"] = xh[c]
        m["xl"] = xl[c]
        in_maps.append(m)

    res = run_bass_kernel_spmd(nc, in_maps, list(range(NCORES)))
    raw = np.concatenate(
        [res.results[c]["out"].astype(np.float32)
         .transpose(0, 1, 3, 2, 4).reshape(BLOC, 2048, NT)
         for c in range(NCORES)], axis=0)
    out = np.maximum(raw * host["fs"][None] + host["fb"][None], 0.0)
    return out.reshape(64, 2048, 16, 16).astype(np.float32)
